# revision 57
# baseline (speedup 1.0000x reference)
"""nn_AttentionOnDetail Trainium2 Bass kernel, 8 NeuronCores.  v3.3

Layout: stage 1 (AFT) is T-sharded; core c owns rows [128c,128(c+1)) of
each global T-half, for both batches.  Per-core tile (128, 4, 1024) with
rt: 0=(b0,H0) 1=(b0,H1) 2=(b1,H0) 3=(b1,H1); chunk c processes rt
{c, c+2} (the AFT batch-reduction pairs them).  Chunk c == global T-half
c, so the stage-2 re-shard A2A fires per (chunk, batch) as soon as that
quarter is normalized; SDPA query-halves gate only on the pieces they
need (causal: queries<1024 need only piece 0).

Queue discipline: the gpsimd queue carries ONLY collectives (plus
programme-top constant builds); a tiny warm-up A2A fires at t~0 to absorb
the CC-runtime startup (~50-150us) under stage-1 compute.  All compute
formerly on gpsimd moved elsewhere:
 - causal diagonal mask = extra accumulating matmul (-300 * strict-lower
   moving tile against a -300*I stationary)
 - softmax denominator broadcast = ones(1,64) x rinv16 matmul (f16)
 - rotary / AFT elementwise on DVE (materialized rotary tables, no
   stride-0 broadcasts).

Stage 1 pre-normalizes (rms per head, fp16 quake-rsqrt bit trick) the
AFT output, so stage 2 consumes ready rows: V' = [u | 1] directly, Q^T
by PE transpose.  Stage-2 rotary cancels (q==k, orthogonal per head).
Stage-1 B-phase is split into head-halves whose serial chains
(square->reduce->rsqrt->rot->apply->exp->AFT->tanh) pipeline across
ACT/DVE; issue order is hand-scheduled so the chunk-0 critical chain
(which gates the first A2A piece) runs ahead of chunk-1 work.

SDPA issue order is software-pipelined: scores(kt+1) issues before
attn(kt) so the PE streams while exp(kt) runs; each exp is split by head
across DVE (Schraudolph bf16 bit-trick, x 2^-9) and ACT (Exp with 2^-9
bias) so the two halves compute concurrently.  Output A2A is split per
(batch, T-half); the mha projection for early pieces overlaps later SDPA
on the PE.  ACT activation-table discipline: all Sin (incl. cos via the
pi/2 bias port) issue before the first Exp -> exactly one table swap.
"""
import sys
import numpy as np

sys.path.insert(0, "/opt/trn_rl_repo")

import concourse.bass as bass
import concourse.mybir as mybir
import concourse.tile as tile
from concourse import bacc
from concourse.bass_utils import run_bass_kernel_spmd
from concourse.masks import make_identity

F32 = mybir.dt.float32
F32R = mybir.dt.float32r
F16 = mybir.dt.float16
BF16 = mybir.dt.bfloat16
I16 = mybir.dt.int16
I32 = mybir.dt.int32
AF = mybir.ActivationFunctionType
ALU = mybir.AluOpType

B, T, C, H, HD = 2, 2048, 1024, 16, 64
NCORES = 8
EPS = 1.1920929e-07
SDPA_SCALE = 0.12
PI = float(np.pi)
MASKC = 300.0
GROUPS = [list(range(NCORES))]
DEBUG = False
NARROW_MASK = True

_CACHE = {}


def _quake_rsqrt(nc, pool, m_ap, n, tag, name=None, out16=None):
    """rf16 = 1/sqrt(m/64) for raw sum-of-64-squares m (128, n) fp32."""
    name = name or tag
    sh = [128, n]
    it = pool.tile(sh, I32, tag=f"{tag}_i", name=f"{name}_i")
    nc.vector.tensor_scalar(it[:], m_ap.bitcast(I32), 1, None,
                            ALU.logical_shift_right)
    sd = pool.tile(sh, I32, tag=f"{tag}_s", name=f"{name}_s")
    nc.vector.tensor_scalar(sd[:], it[:], -1, 0x5F3759DF + 0x01800000,
                            ALU.mult, ALU.add)
    y0 = sd[:].bitcast(F32)
    t = pool.tile(sh, F32, tag=f"{tag}_t", name=f"{name}_t")
    nc.vector.tensor_tensor(t[:], y0, y0, ALU.mult)
    nc.vector.tensor_tensor(t[:], t[:], m_ap, ALU.mult)
    nc.vector.tensor_scalar(t[:], t[:], -0.5 / 64.0, 1.5,
                            ALU.mult, ALU.add)
    rf16 = out16 or pool.tile(sh, F16, tag=f"{tag}_h", name=f"{name}_h")
    nc.vector.tensor_tensor(rf16[:], t[:], y0, ALU.mult)
    return rf16


def _quake_rsqrt16(nc, pool, m_ap, n, tag, name=None):
    """rf16 = 1/sqrt(m/64) for raw sum-of-64-squares m (128, n) fp16.

    fp16 bit-trick: magic 0x59BB + (3<<10) for the /64; one Newton step.
    All 2-byte ops so the DVE fast modes apply.
    """
    name = name or tag
    sh = [128, n]
    it = pool.tile(sh, I16, tag=f"{tag}_i", name=f"{name}_i")
    nc.vector.tensor_scalar(it[:], m_ap.bitcast(I16), 1, None,
                            ALU.logical_shift_right)
    sd = pool.tile(sh, I16, tag=f"{tag}_s", name=f"{name}_s")
    nc.vector.tensor_scalar(sd[:], it[:], -1, 0x59BB + 0x0C00,
                            ALU.mult, ALU.add)
    y0 = sd[:].bitcast(F16)
    t = pool.tile(sh, F16, tag=f"{tag}_t", name=f"{name}_t")
    nc.vector.tensor_tensor(t[:], y0, y0, ALU.mult)
    nc.vector.tensor_tensor(t[:], t[:], m_ap, ALU.mult)
    nc.vector.tensor_scalar(t[:], t[:], -0.5 / 64.0, 1.5,
                            ALU.mult, ALU.add)
    rf16 = pool.tile(sh, F16, tag=f"{tag}_h", name=f"{name}_h")
    nc.vector.tensor_tensor(rf16[:], t[:], y0, ALU.mult)
    return rf16


def build():
    nc = bacc.Bacc("TRN2", target_bir_lowering=False, debug=False,
                   num_devices=NCORES)
    xs_d = nc.dram_tensor("xs", [128, 4, 1024], F32, kind="ExternalInput")
    combw_d = nc.dram_tensor("combw", [128, 24, 128], F16,
                             kind="ExternalInput")
    kvec_d = nc.dram_tensor("kvec", [128, 4], F32, kind="ExternalInput")
    rotc1_d = nc.dram_tensor("rotc1", [128, 2, 16, 16], F16,
                             kind="ExternalInput")
    rots1_d = nc.dram_tensor("rots1", [128, 2, 16, 16], F16,
                             kind="ExternalInput")
    aftT_d = nc.dram_tensor("aftT", [128, 8, 1024], F16,
                            kind="ExternalInput")
    mhaT_d = nc.dram_tensor("mhaT", [128, 8, 1024], F16,
                            kind="ExternalInput")
    out_d = nc.dram_tensor("out", [128, 4, 1024], F32, kind="ExternalOutput")
    if DEBUG:
        dbgu_d = nc.dram_tensor("dbgu", [128, 2, 2, 1024], F16,
                                kind="ExternalOutput")
        dbgy_d = nc.dram_tensor("dbgy", [64, 2, 8, 512], F16,
                                kind="ExternalOutput")
        dbgv_d = nc.dram_tensor("dbgv", [128, 2, 16, 2, 65], BF16,
                                kind="ExternalOutput")
        dbgq_d = nc.dram_tensor("dbgq", [128, 2, 16, 128], F16,
                                kind="ExternalOutput")
        dbgp_d = nc.dram_tensor("dbgp", [65, 2, 512], F32,
                                kind="ExternalOutput")

    with tile.TileContext(nc) as tc, \
         nc.allow_low_precision(reason="fp16 psum softmax accumulators"):
      with tc.tile_pool(name="glob", bufs=1) as gp, \
           tc.tile_pool(name="dram", bufs=1, space="DRAM") as dpool:

        # ---- DRAM bounce buffers for the collectives -------------------
        a2a1_in = [[dpool.tile([NCORES, 128, 128], F16,
                               name=f"a2a1_in{c}{b}") for b in range(2)]
                   for c in range(2)]
        a2a1_out = [[dpool.tile([NCORES, 128, 128], F16,
                                name=f"a2a1_out{c}{b}") for b in range(2)]
                    for c in range(2)]
        a2a2_in = [[dpool.tile([NCORES, 2, 64, 128], F16,
                               name=f"a2a2_in{b}{h}") for h in range(2)]
                   for b in range(2)]
        a2a2_out = [[dpool.tile([NCORES, 2, 64, 128], F16,
                                name=f"a2a2_out{b}{h}") for h in range(2)]
                    for b in range(2)]
        dum_in = dpool.tile([8, 4], F32, name="dum_in")
        dum_out = dpool.tile([8, 4], F32, name="dum_out")

        # ---- constants (gpsimd, before any collective fires) -----------
        ident = gp.tile([128, 128], F16, name="ident")
        make_identity(nc, ident[:])
        b01 = gp.tile([128, 512], F16, name="b01")
        nc.gpsimd.memset(b01[:], 0.0)
        nc.gpsimd.memset(b01[:, 0:128], 1.0)
        # keep 1.0 where p - q - 1 >= 0  (strict lower triangle p > q)
        nc.gpsimd.affine_select(
            out=b01[:, 0:128], in_=b01[:, 0:128], compare_op=ALU.is_ge,
            fill=0.0, base=-1, pattern=[[-1, 128]], channel_multiplier=1)
        negI = gp.tile([128, 128], F16, name="negI")
        nc.vector.tensor_scalar(negI[:], ident[:], -MASKC, None, ALU.mult)
        ones64 = gp.tile([1, 64], F16, name="ones64")
        nc.vector.memset(ones64[:], 1.0)
        # exp scale 2^-9: keeps fp16 PSUM attn accumulators in range
        bias9 = gp.tile([128, 1], F32, name="bias9")
        nc.vector.memset(bias9[:], -6.23832463)

        # ---- CC warm-up: absorb collective-runtime startup -------------
        nc.sync.dma_start(out=dum_in[:], in_=kvec_d[0:8, :])
        nc.gpsimd.collective_compute(
            "AllToAll", ALU.bypass, replica_groups=GROUPS,
            ins=[dum_in[:].opt()], outs=[dum_out[:].opt()])

        u16 = [gp.tile([128, 2, 1024], F16, tag=f"u16_{c}", name=f"u16_{c}")
               for c in range(2)]

        BAS = ("sA", "c1", "m", "p_", "sp", "cp", "mp", "mm")

        # ================= stage 1 (AFT, T-sharded) =====================
        with tc.tile_pool(name="s1", bufs=1) as s1p, \
             tc.tile_pool(name="fe", bufs=2) as fep, \
             tc.tile_pool(name="psA", bufs=2, space="PSUM") as psA, \
             tc.tile_pool(name="pB", bufs=2) as pB, \
             tc.tile_pool(name="psT", bufs=1, space="PSUM") as psT, \
             tc.tile_pool(name="psP", bufs=1, space="PSUM") as psP:

            x = s1p.tile([128, 4, 1024], F32, name="x")
            for c in range(2):
                nc.sync.dma_start(out=x[:, c:c + 3:2, :],
                                  in_=xs_d[:, c:c + 3:2, :])
            kvec = s1p.tile([128, 4], F32, name="kvec")
            nc.sync.dma_start(out=kvec[:], in_=kvec_d[:])
            combw = s1p.tile([128, 24, 128], F16, name="combw")
            nc.sync.dma_start(out=combw[:], in_=combw_d[:])
            rotc1 = s1p.tile([128, 2, 16, 16], F16, name="rotc1")
            rots1 = s1p.tile([128, 2, 16, 16], F16, name="rots1")
            nc.sync.dma_start(out=rotc1[:], in_=rotc1_d[:])
            nc.sync.dma_start(out=rots1[:], in_=rots1_d[:])
            aftw = s1p.tile([128, 8, 1024], F16, name="aftw")
            nc.sync.dma_start(out=aftw[:], in_=aftT_d[:])

            qkv = [[s1p.tile([128, 2, 1024], F16, name=f"qkv{i}_{c}")
                    for i in range(3)] for c in range(2)]

            # cos(pi*h) = sin(pi*h + pi/2) via the activation bias port
            hpi = s1p.tile([128, 1], F32, name="hpi")
            nc.vector.memset(hpi[:], PI / 2)

            basis = [None, None]

            def _basis_act(c):
                xc = x[:, c:c + 3:2, :]
                h = fep.tile([128, 2, 1024], F16, tag="h", name=f"h{c}")
                nc.scalar.activation(h[:], xc, AF.Tanh, scale=0.5)
                bs = {nm: fep.tile([128, 2, 1024], F16, tag=nm,
                                   name=f"{nm}{c}") for nm in BAS}
                basis[c] = bs
                nc.scalar.activation(bs["sA"][:], h[:], AF.Sin, scale=PI)
                nc.scalar.activation(bs["c1"][:], h[:], AF.Sin,
                                     scale=PI, bias=hpi[:, 0:1])
                nc.scalar.activation(bs["p_"][:], bs["sA"][:], AF.Square)

            def _basis_m(c):
                bs = basis[c]
                nc.vector.tensor_tensor(bs["m"][:], bs["sA"][:],
                                        bs["c1"][:], ALU.mult)

            def _basis_rest(c):
                bs = basis[c]
                nc.scalar.activation(bs["mm"][:], bs["m"][:], AF.Square)
                nc.vector.tensor_tensor(bs["sp"][:], bs["sA"][:],
                                        bs["p_"][:], ALU.mult)
                nc.vector.tensor_tensor(bs["cp"][:], bs["c1"][:],
                                        bs["p_"][:], ALU.mult)
                nc.vector.tensor_tensor(bs["mp"][:], bs["m"][:],
                                        bs["p_"][:], ALU.mult)

            def _combine(c):
                bs = basis[c]
                for pc in range(4):
                    rs, c0 = pc // 2, 512 * (pc % 2)
                    pss = [psA.tile([128, 512], F32, tag=f"c{i}",
                                    name=f"c{i}") for i in range(3)]
                    for f in range(8):
                        mv = bs[BAS[f]][:, rs, c0:c0 + 512]
                        for i in range(3):
                            nc.tensor.matmul(
                                pss[i][:], combw[:, 8 * i + f, :], mv,
                                start=(f == 0), stop=(f == 7))
                    dst = [qkv[c][i][:, rs, c0:c0 + 512] for i in range(3)]
                    nc.scalar.activation(dst[0], pss[0][:], AF.Identity,
                                         bias=kvec[:, 0:1])
                    if pc % 2 == 0:
                        nc.vector.tensor_scalar(dst[1], pss[1][:],
                                                kvec[:, 1:2], None, ALU.add)
                    else:
                        nc.scalar.activation(dst[1], pss[1][:], AF.Identity,
                                             bias=kvec[:, 1:2])
                    nc.scalar.activation(dst[2], pss[2][:], AF.Identity,
                                         bias=kvec[:, 2:3])

            rfs = [[[None] * 2 for _ in range(3)] for _ in range(2)]
            rtls = [None, None]

            # B-phase ops operate on a head-half g (heads 8g..8g+7 =
            # channel cols 512g..512g+512); the two halves pipeline
            # across ACT and DVE, halving the serial-chain latency.
            def _stats(c, i, g):
                cs = 512 * g
                sq = pB.tile([128, 2, 512], F16, tag="sq",
                             name=f"sq{i}_{c}{g}")
                nc.scalar.activation(
                    sq[:], qkv[c][i][:, :, cs:cs + 512], AF.Square)
                ssq = pB.tile([128, 16], F16, tag=f"ssq{i}{g}",
                              name=f"ssq{i}_{c}{g}")
                nc.vector.tensor_reduce(
                    ssq[:],
                    sq[:].rearrange("p a (h d) -> p (a h) d", h=8),
                    axis=mybir.AxisListType.X, op=ALU.add)
                rfs[c][i][g] = _quake_rsqrt16(nc, pB, ssq[:], 16,
                                              f"rf{i}{g}", f"rf{i}_{c}{g}")

            def _rot(c, i, g):
                qv = qkv[c][i][:].rearrange("p a (h d) -> p a h d", h=16)
                hs = 8 * g
                x1 = qv[:, :, hs:hs + 8, 0:16]
                x2 = qv[:, :, hs:hs + 8, 32:48]
                rc = rotc1[:, :, hs:hs + 8, :]
                rs_ = rots1[:, :, hs:hs + 8, :]
                u1 = pB.tile([128, 2, 8, 16], F16, tag="ru1",
                             name=f"ru1{i}_{c}{g}")
                u2 = pB.tile([128, 2, 8, 16], F16, tag="ru2",
                             name=f"ru2{i}_{c}{g}")
                t1 = pB.tile([128, 2, 8, 16], F16, tag="rt1",
                             name=f"rt1{i}_{c}{g}")
                nc.vector.tensor_tensor(u1[:], x2, rs_, ALU.mult)
                nc.vector.tensor_tensor(u2[:], x1, rs_, ALU.mult)
                nc.vector.tensor_tensor(t1[:], x1, rc, ALU.mult)
                nc.vector.tensor_tensor(x1, t1[:], u1[:], ALU.add)
                nc.vector.tensor_tensor(t1[:], x2, rc, ALU.mult)
                nc.vector.tensor_tensor(x2, t1[:], u2[:], ALU.subtract)

            def _apply(c, i, g):
                rb = rfs[c][i][g][:].rearrange("p (a h) -> p a h", a=2) \
                    .unsqueeze(3).broadcast_to([128, 2, 8, 64])
                hs = 8 * g
                v4 = qkv[c][i][:].rearrange("p a (h d) -> p a h d",
                                            h=16)[:, :, hs:hs + 8, :]
                nc.vector.tensor_tensor(v4, v4, rb, ALU.mult)

            def _aft_head(c, g):
                cs = 512 * g
                ek = qkv[c][1][:, :, cs:cs + 512]
                nc.scalar.activation(ek, ek, AF.Exp)
                s_ = pB.tile([128, 512], F32, tag=f"s_{g}",
                             name=f"s_{c}{g}")
                nc.vector.tensor_tensor(s_[:], ek[:, 0, :], ek[:, 1, :],
                                        ALU.add)
                sinv = pB.tile([128, 512], F32, tag=f"sinv{g}",
                               name=f"sinv{c}{g}")
                nc.vector.reciprocal_approx_fast(sinv[:], s_[:])
                t0 = pB.tile([128, 512], F16, tag=f"t0{g}",
                             name=f"t0{c}{g}")
                t1_ = pB.tile([128, 512], F16, tag=f"t1{g}",
                              name=f"t1{c}{g}")
                v_ = qkv[c][2][:, :, cs:cs + 512]
                nc.vector.tensor_tensor(t0[:], ek[:, 0, :], v_[:, 0, :],
                                        ALU.mult)
                nc.vector.tensor_tensor(t1_[:], ek[:, 1, :], v_[:, 1, :],
                                        ALU.mult)
                nc.vector.tensor_tensor(t0[:], t0[:], t1_[:], ALU.add)
                r_ = pB.tile([128, 512], F16, tag=f"r_{g}",
                             name=f"r_{c}{g}")
                nc.vector.tensor_tensor(r_[:], t0[:], sinv[:], ALU.mult)
                return r_

            def _aft_tail(c, g, r_):
                cs = 512 * g
                tq = qkv[c][0][:, :, cs:cs + 512]
                nc.scalar.activation(tq, tq, AF.Tanh, scale=0.5)
                y1 = qkv[c][2][:, :, cs:cs + 512]
                rb_ = r_[:].unsqueeze(1).broadcast_to([128, 2, 512])
                nc.vector.scalar_tensor_tensor(
                    y1, tq, 1.0, rb_, ALU.add, ALU.mult)

            def _transposes(c, g):
                y1 = qkv[c][2]
                if rtls[c] is None:
                    rtls[c] = pB.tile([128, 8, 256], F16, tag="y1T",
                                      name=f"y1T{c}")
                y1T = rtls[c]
                for rs in range(2):
                    for cp2 in range(2 * g, 2 * g + 2):
                        pst = psT.tile([128, 256], F16, tag="pst",
                                       name="pst")
                        for k2 in range(2):
                            cb8 = 2 * cp2 + k2
                            nc.tensor.transpose(
                                pst[:, 128 * k2:128 * (k2 + 1)],
                                y1[:, rs, 128 * cb8:128 * (cb8 + 1)],
                                ident[:])
                        nc.vector.tensor_copy(
                            y1T[:, 2 * cp2:2 * cp2 + 2,
                                128 * rs:128 * (rs + 1)],
                            pst[:].rearrange("p (a b) -> p a b", a=2))

            def _projnorm(c, rs):
                # projection + rms-norm + ship for batch rs of chunk c
                y1T = rtls[c]
                a16 = pB.tile([128, 1024], F16, tag=f"a16{rs}",
                              name=f"a16{c}{rs}")
                for oc in range(2):
                    pa = psP.tile([128, 512], F32, tag="pa", name="pa")
                    for cb8 in range(8):
                        nc.tensor.matmul(
                            pa[:],
                            y1T[:, cb8, 128 * rs:128 * (rs + 1)],
                            aftw[:, cb8, 512 * oc:512 * (oc + 1)],
                            start=(cb8 == 0), stop=(cb8 == 7))
                    nc.scalar.activation(
                        a16[:, 512 * oc:512 * (oc + 1)], pa[:],
                        AF.Identity)
                asq = pB.tile([128, 1024], F16, tag=f"asq{rs}",
                              name=f"asq{c}{rs}")
                nc.vector.tensor_tensor(asq[:], a16[:], a16[:], ALU.mult)
                assq = pB.tile([128, 16], F16, tag=f"assq{rs}",
                               name=f"assq{c}{rs}")
                nc.vector.tensor_reduce(
                    assq[:],
                    asq[:].rearrange("p (h d) -> p h d", h=16),
                    axis=mybir.AxisListType.X, op=ALU.add)
                arf = _quake_rsqrt16(nc, pB, assq[:], 16, f"arf{rs}",
                                     f"arf{c}{rs}")
                arb = arf[:].unsqueeze(2).broadcast_to([128, 16, 64])
                nc.vector.tensor_tensor(
                    u16[c][:, rs, :].rearrange("p (h d) -> p h d", h=16),
                    a16[:].rearrange("p (h d) -> p h d", h=16),
                    arb, ALU.mult)
                nc.sync.dma_start(
                    out=a2a1_in[c][rs][:].rearrange("d p c2 -> p d c2"),
                    in_=u16[c][:, rs, :].rearrange(
                        "p (d c2) -> p d c2", d=8))
                nc.gpsimd.collective_compute(
                    "AllToAll", ALU.bypass, replica_groups=GROUPS,
                    ins=[a2a1_in[c][rs][:].opt()],
                    outs=[a2a1_out[c][rs][:].opt()])
                if DEBUG:
                    nc.sync.dma_start(out=dbgu_d[:, c, rs],
                                      in_=u16[c][:, rs, :])

            def _bphase(c):
                # two head-half chains, interleaved so ACT(g1) overlaps
                # DVE(g0) at every step
                for g in range(2):
                    _stats(c, 1, g)
                for g in range(2):
                    _rot(c, 1, g)
                    _apply(c, 1, g)
                for g in range(2):
                    _stats(c, 2, g)
                    _apply(c, 2, g)
                r = [None, None]
                for g in range(2):
                    r[g] = _aft_head(c, g)
                for g in range(2):
                    _stats(c, 0, g)
                    _rot(c, 0, g)
                    _apply(c, 0, g)
                for g in range(2):
                    _aft_tail(c, g, r[g])
                    _transposes(c, g)
                _projnorm(c, 0)
                _projnorm(c, 1)

            # ---- hand-scheduled issue order: chunk-0 chain first -------
            _basis_act(0)
            _basis_m(0)
            _basis_rest(0)
            _combine(0)
            _stats(0, 1, 0)              # k chain c0 half-0 first
            _basis_act(1)                # c1 sins overlap c0 k chain
            _basis_m(1)
            _stats(0, 1, 1)
            _rot(0, 1, 0)
            _apply(0, 1, 0)
            _basis_rest(1)
            _rot(0, 1, 1)
            _apply(0, 1, 1)
            _stats(0, 2, 0)
            _apply(0, 2, 0)
            _stats(0, 2, 1)
            _apply(0, 2, 1)
            r00 = _aft_head(0, 0)        # exp: after ALL sins (table swap)
            r01 = _aft_head(0, 1)
            _stats(0, 0, 0)
            _rot(0, 0, 0)
            _apply(0, 0, 0)
            _stats(0, 0, 1)
            _rot(0, 0, 1)
            _apply(0, 0, 1)
            _aft_tail(0, 0, r00)
            _transposes(0, 0)
            _aft_tail(0, 1, r01)
            _transposes(0, 1)
            _combine(1)
            _projnorm(0, 0)
            _projnorm(0, 1)
            _bphase(1)

        # ============ stage 2: causal SDPA + stage 3 projection =========
        with tc.tile_pool(name="pE", bufs=1) as pE, \
             tc.tile_pool(name="psE", bufs=2, space="PSUM") as psE, \
             tc.tile_pool(name="psY", bufs=1, space="PSUM") as psY, \
             tc.tile_pool(name="psQ", bufs=1, space="PSUM") as psQ, \
             tc.tile_pool(name="psRH", bufs=1, space="PSUM") as psRH, \
             tc.tile_pool(name="pe16", bufs=4) as pe16, \
             tc.tile_pool(name="pH", bufs=2) as pH:
            out_sb = pE.tile([128, 4, 1024], F32, tag="out_sb",
                             name="out_sb")
            mhaw = pE.tile([128, 8, 1024], F16, tag="mhaw", name="mhaw")
            nc.scalar.dma_start(out=mhaw[:], in_=mhaT_d[:])

            QT = [pE.tile([128, 16, 128], F16, tag=f"QT_{b}",
                          name=f"QT_{b}") for b in range(2)]
            V16 = [pE.tile([128, 16, 2, 65], BF16, tag=f"V16_{b}",
                           name=f"V16_{b}") for b in range(2)]
            Y16 = [pE.tile([64, 8, 512], F16, tag=f"Y16_{b}",
                           name=f"Y16_{b}") for b in range(2)]

            def _preamble(b, piece):
                A2 = pE.tile([128, 8, 128], F16, tag=f"A2_{b}{piece}",
                             name=f"A2_{b}{piece}")
                nc.scalar.dma_start(
                    out=A2[:],
                    in_=a2a1_out[piece][b][:].rearrange("s p c -> p s c"))
                nc.vector.tensor_copy(
                    V16[b][:, 8 * piece:8 * piece + 8, :, 0:64],
                    A2[:].rearrange("p s (hh d) -> p s hh d", hh=2))
                if piece == 0:
                    nc.vector.memset(V16[b][:, :, :, 64:65], 1.0)
                for jq in range(2):
                    pst2 = psQ.tile([128, 512], F16, tag="pst2",
                                    name="pst2")
                    for k4 in range(4):
                        j = 4 * jq + k4
                        nc.tensor.transpose(
                            pst2[:, 128 * k4:128 * (k4 + 1)],
                            A2[:, j, :], ident[:])
                    nc.vector.tensor_copy(
                        QT[b][:, 8 * piece + 4 * jq:
                              8 * piece + 4 * (jq + 1), :],
                        pst2[:].rearrange("p (a b) -> p a b", a=4))

            def _sdpa_qc(b, qc):
                nkt = 4 * qc + 4
                qtf = QT[b][:].rearrange("p a b -> p (a b)")
                pend = []          # (kt, ps_s, e16) awaiting attn issue

                def _issue_attn(kt, e16, off):
                    for hh in range(2):
                        nc.tensor.matmul(
                            pys[hh][:, off:512],
                            V16[b][:, kt, hh, :],
                            e16[:, hh, off:512],
                            start=(kt == 0), stop=(kt == nkt - 1))

                pys = [psY.tile([65, 512], F32, tag=f"py{hh}",
                                name=f"py{hh}") for hh in range(2)]
                for kt in range(nkt):
                    off = max(0, 128 * (kt - 4 * qc))
                    ps_s = psE.tile([128, 2, 512], F32, tag="ps_s",
                                    name="ps_s")
                    diag = kt >= 4 * qc
                    for hh in range(2):
                        hb = 64 * hh
                        nc.tensor.matmul(
                            ps_s[:, hh, off:512],
                            qtf[hb:hb + 64, 128 * kt:128 * (kt + 1)],
                            qtf[hb:hb + 64, 512 * qc + off:512 * (qc + 1)],
                            start=True, stop=not diag)
                        if diag:
                            if NARROW_MASK:
                                nc.tensor.matmul(
                                    ps_s[:, hh, off:off + 128],
                                    negI[:], b01[:, 0:128],
                                    start=False, stop=True)
                            else:
                                nc.tensor.matmul(
                                    ps_s[:, hh, off:512],
                                    negI[:], b01[:, 0:512 - off],
                                    start=False, stop=True)
                    e16 = pe16.tile([128, 2, 512], BF16, tag="e16",
                                    name="e16")
                    # halves run concurrently: DVE Schraudolph + ACT Exp
                    nc.vector.tensor_scalar(
                        e16[:, 0, off:512].bitcast(I16),
                        ps_s[:, 0, off:512],
                        22.159803, 15093.5, ALU.mult, ALU.add)
                    nc.scalar.activation(e16[:, 1, off:512],
                                         ps_s[:, 1, off:512], AF.Exp,
                                         scale=SDPA_SCALE,
                                         bias=bias9[:, 0:1])
                    pend.append((kt, e16, off))
                    if len(pend) > 1:
                        _issue_attn(*pend.pop(0))
                _issue_attn(*pend.pop(0))

                if DEBUG and b == 0 and qc == 0:
                    for hh in range(2):
                        pd = pE.tile([65, 512], F32, tag=f"dbgp{hh}",
                                     name=f"dbgp{hh}")
                        nc.vector.tensor_copy(pd[:], pys[hh][:])
                        nc.sync.dma_start(out=dbgp_d[:, hh, :], in_=pd[:])
                for hh in range(2):
                    den = pE.tile([1, 512], F32, tag="den",
                                  name=f"den{b}{qc}{hh}")
                    nc.scalar.activation(den[:], pys[hh][64:65, :],
                                         AF.Identity)
                    rinv = pE.tile([1, 512], F32, tag="rinv",
                                   name=f"rinv{b}{qc}{hh}")
                    nc.vector.reciprocal_approx_fast(rinv[:], den[:])
                    rinv16 = pE.tile([1, 512], F16, tag="rinv16",
                                     name=f"rinv16{b}{qc}{hh}")
                    nc.vector.tensor_copy(rinv16[:], rinv[:])
                    psR = psRH.tile([128, 512], F32, tag="psR", name="psR")
                    nc.tensor.matmul(
                        psR[0:64, :], ones64[:], rinv16[:],
                        start=True, stop=True)
                    RSR = pE.tile([64, 512], F16, tag="RSR",
                                  name=f"RSR{b}{qc}{hh}")
                    nc.scalar.activation(RSR[:], psR[0:64, :], AF.Identity)
                    nc.vector.tensor_tensor(
                        Y16[b][:, 4 * hh + qc, :], pys[hh][0:64, :],
                        RSR[:], ALU.mult)
                    # ship Y rows: dest core c2 owns q rows
                    # [1024*(qc//2) + 128*c2, +128)
                    u = qc % 2
                    nc.sync.dma_start(
                        out=a2a2_in[b][qc // 2][4 * u:4 * u + 4, hh, :, :]
                            .rearrange("d p q -> p d q"),
                        in_=Y16[b][:, 4 * hh + qc, :].rearrange(
                            "p (d q) -> p d q", d=4))

            def _stage3(b, hp):
                YF = pH.tile([128, 8, 128], F16, tag="YF", name=f"YF{b}{hp}")
                nc.scalar.dma_start(
                    out=YF[:],
                    in_=a2a2_out[b][hp][:].rearrange(
                        "s hh p q -> (hh p) s q"))
                rt = 2 * b + hp
                for oc in range(2):
                    pm = psRH.tile([128, 512], F32, tag="psR", name="pm")
                    for s_ in range(NCORES):
                        nc.tensor.matmul(
                            pm[:],
                            YF[:, s_, :],
                            mhaw[:, s_, 512 * oc:512 * (oc + 1)],
                            start=(s_ == 0), stop=(s_ == 7))
                    nc.vector.tensor_copy(
                        out_sb[:, rt, 512 * oc:512 * (oc + 1)], pm[:])
                nc.sync.dma_start(out=out_d[:, rt:rt + 1, :],
                                  in_=out_sb[:, rt:rt + 1, :])

            # ---- b = 0 --------------------------------------------------
            _preamble(0, 0)
            _sdpa_qc(0, 0)
            _sdpa_qc(0, 1)
            nc.gpsimd.collective_compute(
                "AllToAll", ALU.bypass, replica_groups=GROUPS,
                ins=[a2a2_in[0][0][:].opt()], outs=[a2a2_out[0][0][:].opt()])
            _preamble(0, 1)
            _sdpa_qc(0, 2)
            _sdpa_qc(0, 3)
            nc.gpsimd.collective_compute(
                "AllToAll", ALU.bypass, replica_groups=GROUPS,
                ins=[a2a2_in[0][1][:].opt()], outs=[a2a2_out[0][1][:].opt()])
            # ---- b = 1 preamble + early stage-3 pieces ------------------
            _preamble(1, 0)
            _preamble(1, 1)
            _stage3(0, 0)
            _sdpa_qc(1, 0)
            _stage3(0, 1)
            _sdpa_qc(1, 1)
            nc.gpsimd.collective_compute(
                "AllToAll", ALU.bypass, replica_groups=GROUPS,
                ins=[a2a2_in[1][0][:].opt()], outs=[a2a2_out[1][0][:].opt()])
            _sdpa_qc(1, 2)
            _sdpa_qc(1, 3)
            nc.gpsimd.collective_compute(
                "AllToAll", ALU.bypass, replica_groups=GROUPS,
                ins=[a2a2_in[1][1][:].opt()], outs=[a2a2_out[1][1][:].opt()])
            _stage3(1, 0)
            _stage3(1, 1)
            if DEBUG:
                for b in range(2):
                    nc.sync.dma_start(out=dbgy_d[:, b], in_=Y16[b][:])
                    nc.sync.dma_start(out=dbgv_d[:, b], in_=V16[b][:])
                    nc.sync.dma_start(out=dbgq_d[:, b], in_=QT[b][:])

    nc.compile()
    return nc


def _host_inputs(x, kqv, c_proj):
    """Build per-core input maps from the full problem inputs."""
    A = kqv[:, :5].astype(np.float64)
    Bc = kqv[:, 5:].astype(np.float64)
    coef = np.zeros((8, 3), np.float64)
    K = A[:, 0] + Bc[:, 0] + Bc[:, 2] + Bc[:, 4]
    coef[0] = A[:, 1] + 3.0 * A[:, 3]
    coef[1] = Bc[:, 1] + Bc[:, 3]
    coef[2] = 2.0 * A[:, 2] + 4.0 * A[:, 4]
    coef[3] = -2.0 * Bc[:, 2]
    coef[4] = -4.0 * A[:, 3]
    coef[5] = -4.0 * Bc[:, 3]
    coef[6] = -8.0 * A[:, 4]
    coef[7] = -8.0 * Bc[:, 4]

    eye = np.eye(128, dtype=np.float32)
    combw = np.zeros((128, 24, 128), np.float16)
    for i in range(3):
        for f in range(8):
            combw[:, 8 * i + f, :] = (eye * np.float32(coef[f, i])) \
                .astype(np.float16)
    kvec = np.zeros((128, 4), np.float32)
    kvec[:, :3] = K.astype(np.float32)[None, :]

    freq = (1.0 / 1024.0) ** np.linspace(0.0, 1.0, 16, dtype=np.float32)
    hh = np.arange(16, dtype=np.float32)
    theta = np.outer(hh, freq)
    rotc1 = np.broadcast_to(np.cos(theta).astype(np.float16),
                            (128, 2, 16, 16)).copy()
    rots1 = np.broadcast_to(np.sin(theta).astype(np.float16),
                            (128, 2, 16, 16)).copy()

    W1 = c_proj[:, :C]
    W2 = c_proj[:, C:]
    aftT = (0.5 * W1.T).reshape(8, 128, 1024).transpose(1, 0, 2) \
        .astype(np.float16).copy()
    mhaT = W2.T.reshape(8, 128, 1024).transpose(1, 0, 2) \
        .astype(np.float16).copy()

    in_maps = []
    for c in range(NCORES):
        r0 = 128 * c
        xs = np.stack([x[0, r0:r0 + 128, :],
                       x[0, 1024 + r0:1024 + r0 + 128, :],
                       x[1, r0:r0 + 128, :],
                       x[1, 1024 + r0:1024 + r0 + 128, :]], axis=1)
        in_maps.append(dict(xs=np.ascontiguousarray(xs), combw=combw,
                            kvec=kvec, rotc1=rotc1, rots1=rots1,
                            aftT=aftT, mhaT=mhaT))
    return in_maps


def kernel(x, kqv, c_proj):
    x = np.asarray(x, np.float32)
    kqv = np.asarray(kqv, np.float32)
    c_proj = np.asarray(c_proj, np.float32)
    if "nc" not in _CACHE:
        _CACHE["nc"] = build()
    nc = _CACHE["nc"]
    in_maps = _host_inputs(x, kqv, c_proj)
    res = run_bass_kernel_spmd(nc, in_maps, core_ids=list(range(NCORES)))
    out = np.empty((B, T, C), np.float32)
    for c in range(NCORES):
        oc = res.results[c]["out"]          # (128, 4, 1024)
        r0 = 128 * c
        for b in range(2):
            for hp in range(2):
                out[b, 1024 * hp + r0:1024 * hp + r0 + 128, :] = \
                    oc[:, 2 * b + hp, :]
    return out
